# revision 2
# baseline (speedup 1.0000x reference)
"""nn_MC_GRU Trainium2 kernel: 8-core SPMD Bass/Tile implementation.

Sharding: 4 feature-groups x 2 batch-halves (one 4-feature group of
independent GRUs per core, 512 batch rows per core). Layout on device is
[128 partitions = 4 features x 32 hidden, B free]. Per timestep, three
matmul-pairs (bf16 inputs, fp32 psum) produce the gate preactivations
(biases folded in via an ones-row in the packed input; the tanh gate is
computed as 2*sigmoid(2x)-1 with the 2x folded into the weights); both
sigmoids run in a single ACT op; the cell update runs on DVE/GPSIMD.

Ragged sequences: the batch is sorted by length (descending) and dealt
across cores/chunks, and the compiled schedule processes only the active
prefix at each step (widths derived from the actual lens at first call —
the program is rebuilt if lens change). Rows past their length are
additionally frozen exactly by a mask row that drives the update-gate
sigmoid to 0. State H = h+1 in fp32 with a bf16 shadow feeding matmuls.

The tiny MLP head (0.5% of FLOPs) runs on host. Output matches the
reference within ~5e-3 relative error (bf16 matmul rounding).
"""

import numpy as np
import ml_dtypes

from concourse import bacc, tile, mybir, bass_utils

F, D, HD, B, T = 16, 4, 32, 1024, 512
N_CORES = 8
MASK_VAL = 88.0
XROWS = 18
MU = 2
CH = 32

f32 = mybir.dt.float32
bf16 = mybir.dt.bfloat16
AF = mybir.ActivationFunctionType
OP = mybir.AluOpType
BF16NP = ml_dtypes.bfloat16


def _build_nc(widths, mu=MU, ch=CH):
    b_mu = 512 // mu
    T_steps = len(widths)
    mmdt = bf16
    nc = bacc.Bacc("TRN2", target_bir_lowering=False, debug=False,
                   num_devices=N_CORES)
    xp = nc.dram_tensor("xp", [XROWS, T, 512], mmdt, kind="ExternalInput").ap()
    drs = {}
    for nm in ("lr", "lzb", "ln2"):
        drs[nm] = nc.dram_tensor(nm, [128, 128], mmdt, kind="ExternalInput").ap()
    for nm in ("gr", "gzb", "gn2"):
        drs[nm] = nc.dram_tensor(nm, [XROWS, 128], mmdt, kind="ExternalInput").ap()
    bhn2 = nc.dram_tensor("bhn2", [128, 1], f32, kind="ExternalInput").ap()
    hout = nc.dram_tensor("hout", [128, 512], f32, kind="ExternalOutput").ap()

    n_chunks = (T_steps + ch - 1) // ch
    with tile.TileContext(nc) as tc:
        with (
            tc.tile_pool(name="wpool", bufs=1) as wpool,
            tc.tile_pool(name="hpool", bufs=1) as hpool,
            tc.tile_pool(name="xpool", bufs=2) as xpool,
            tc.tile_pool(name="work", bufs=2) as work,
            tc.tile_pool(name="psum", bufs=2, space="PSUM") as pp,
        ):
            wL = {}
            for nm, shp in (("lr", [128, 128]), ("lzb", [128, 128]),
                            ("ln2", [128, 128]), ("gr", [XROWS, 128]),
                            ("gzb", [XROWS, 128]), ("gn2", [XROWS, 128])):
                t_ = wpool.tile(shp, mmdt, tag=nm, name=nm)
                nc.sync.dma_start(t_[:], drs[nm][:])
                wL[nm] = t_
            t_ = wpool.tile([128, 1], f32, tag="bhn2", name="bhn2")
            nc.sync.dma_start(t_[:], bhn2[:])
            wL["bhn2"] = t_

            Hm32 = [hpool.tile([128, b_mu], f32, tag=f"Hm{m}", name=f"Hm{m}")
                    for m in range(mu)]
            Hsh = [hpool.tile([128, b_mu], mmdt, tag=f"Hb{m}", name=f"Hb{m}")
                   for m in range(mu)]
            for m in range(mu):
                nc.vector.memset(Hm32[m][:], 1.0)
                nc.vector.memset(Hsh[m][:], 1.0)

            for c in range(n_chunks):
                t0 = c * ch
                t1 = min(t0 + ch, T_steps)
                xt_tile = xpool.tile([XROWS, t1 - t0, 512], mmdt, tag="x",
                                     name="xt_tile")
                nc.sync.dma_start(xt_tile[:], xp[:, t0:t1, :])
                for t in range(t0, t1):
                    w = widths[t]
                    if w <= 0:
                        continue
                    P, WK = [], []
                    for m in range(mu):
                        p_all = pp.tile([128, 4 * b_mu], f32, tag=f"pa{m}",
                                        name=f"pa{m}")
                        P.append(dict(
                            r=p_all[:, 0:w],
                            zb=p_all[:, w:2 * w],
                            rzb=p_all[:, 0:2 * w],
                            hn=p_all[:, 2 * b_mu:2 * b_mu + w],
                            inn=p_all[:, 3 * b_mu:3 * b_mu + w]))
                        WK.append({nm: work.tile(
                            [128, 2 * b_mu] if nm == "srzb" else [128, b_mu],
                            f32, tag=f"{nm}{m}", name=f"{nm}{m}")
                            for nm in ("srzb", "u", "sn", "p1", "g", "v")})
                    for m in range(mu):
                        xsl = xt_tile[:, t - t0, m * b_mu:m * b_mu + w]
                        nc.tensor.matmul(P[m]["inn"], wL["gn2"][:], xsl,
                                         start=True, stop=True)
                        nc.tensor.matmul(P[m]["r"], wL["gr"][:], xsl,
                                         start=True, stop=False)
                        nc.tensor.matmul(P[m]["zb"], wL["gzb"][:], xsl,
                                         start=True, stop=False)
                    for m in range(mu):
                        Hsl = Hsh[m][:, 0:w]
                        nc.tensor.matmul(P[m]["r"], wL["lr"][:], Hsl,
                                         start=False, stop=True)
                        nc.tensor.matmul(P[m]["zb"], wL["lzb"][:], Hsl,
                                         start=False, stop=True)
                        nc.tensor.matmul(P[m]["hn"], wL["ln2"][:], Hsl,
                                         start=True, stop=True)
                    for m in range(mu):
                        nc.scalar.activation(WK[m]["srzb"][:, 0:2 * w],
                                             P[m]["rzb"], AF.Sigmoid)
                    for m in range(mu):
                        nc.vector.scalar_tensor_tensor(
                            WK[m]["u"][:, 0:w], P[m]["hn"], wL["bhn2"][:],
                            WK[m]["srzb"][:, 0:w], op0=OP.add, op1=OP.mult)
                    for m in range(mu):
                        nc.gpsimd.tensor_tensor(
                            WK[m]["p1"][:, 0:w], WK[m]["srzb"][:, w:2 * w],
                            Hm32[m][:, 0:w], op=OP.mult)
                    for m in range(mu):
                        nc.vector.tensor_tensor(
                            P[m]["inn"], WK[m]["u"][:, 0:w],
                            P[m]["inn"], op=OP.add)
                    for m in range(mu):
                        nc.gpsimd.tensor_tensor(
                            WK[m]["g"][:, 0:w], Hm32[m][:, 0:w],
                            WK[m]["p1"][:, 0:w], op=OP.subtract)
                    for m in range(mu):
                        nc.scalar.activation(WK[m]["sn"][:, 0:w],
                                             P[m]["inn"], AF.Sigmoid)
                    for m in range(mu):
                        nc.vector.scalar_tensor_tensor(
                            WK[m]["v"][:, 0:w], WK[m]["sn"][:, 0:w], 2.0,
                            WK[m]["srzb"][:, w:2 * w], op0=OP.mult,
                            op1=OP.mult)
                    for m in range(mu):
                        nc.vector.tensor_tensor(
                            Hsh[m][:, 0:w], WK[m]["g"][:, 0:w],
                            WK[m]["v"][:, 0:w], op=OP.add)
                    for m in range(mu):
                        nc.gpsimd.tensor_tensor(
                            Hm32[m][:, 0:w], WK[m]["g"][:, 0:w],
                            WK[m]["v"][:, 0:w], op=OP.add)

            for m in range(mu):
                nc.sync.dma_start(hout[:, m * b_mu:(m + 1) * b_mu],
                                  Hm32[m][:])
    nc.compile()
    return nc


def _prep(inputs, mu=MU):
    x = np.asarray(inputs["input"], np.float32)
    lens = np.asarray(inputs["lens"]).astype(np.int64)
    W_ih = np.asarray(inputs["W_ih"], np.float32)
    W_hh = np.asarray(inputs["W_hh"], np.float32)
    b_ih = np.asarray(inputs["b_ih"], np.float32)
    b_hh = np.asarray(inputs["b_hh"], np.float32)

    order = np.argsort(-lens, kind="stable")
    halves = [order[0::2], order[1::2]]
    halves = [np.concatenate([h[m::mu] for m in range(mu)]) for h in halves]

    b_mu = 512 // mu
    widths = []
    for t in range(T):
        k = max(int((lens[h[0:b_mu]] > t).sum()) for h in halves)
        widths.append(min(b_mu, ((k + 7) // 8) * 8))

    tabs = {}
    for fgrp in range(4):
        fs = np.arange(4 * fgrp, 4 * fgrp + 4)
        L_r = np.zeros((128, 128), np.float32)
        L_zb = np.zeros((128, 128), np.float32)
        L_n2 = np.zeros((128, 128), np.float32)
        G_r = np.zeros((XROWS, 128), np.float32)
        G_zb = np.zeros((XROWS, 128), np.float32)
        G_n2 = np.zeros((XROWS, 128), np.float32)
        b_hn2 = np.zeros((128, 1), np.float32)
        for fl, f in enumerate(fs):
            sl = slice(32 * fl, 32 * fl + 32)
            rsl = slice(4 * fl, 4 * fl + 4)
            Wr, Wz, Wn = (W_hh[f, 0:32], W_hh[f, 32:64], W_hh[f, 64:96])
            Ur, Uz, Un = (W_ih[f, 0:32], W_ih[f, 32:64], W_ih[f, 64:96])
            L_r[sl, sl] = Wr.T
            L_zb[sl, sl] = -Wz.T
            L_n2[sl, sl] = 2.0 * Wn.T
            G_r[rsl, sl] = Ur.T
            G_zb[rsl, sl] = -Uz.T
            G_n2[rsl, sl] = 2.0 * Un.T
            G_zb[16, sl] = -1.0
            G_r[17, sl] = b_ih[f, 0:32] + b_hh[f, 0:32] - Wr.sum(1)
            G_zb[17, sl] = -(b_ih[f, 32:64] + b_hh[f, 32:64]) + Wz.sum(1)
            G_n2[17, sl] = 2.0 * b_ih[f, 64:96]
            b_hn2[sl, 0] = 2.0 * (b_hh[f, 64:96] - Wn.sum(1))
        tabs[fgrp] = dict(lr=L_r.astype(BF16NP), lzb=L_zb.astype(BF16NP),
                          ln2=L_n2.astype(BF16NP), gr=G_r.astype(BF16NP),
                          gzb=G_zb.astype(BF16NP), gn2=G_n2.astype(BF16NP),
                          bhn2=b_hn2)

    in_maps = []
    for c in range(N_CORES):
        fgrp, bhalf = c % 4, c // 4
        cols = halves[bhalf]
        fs = np.arange(4 * fgrp, 4 * fgrp + 4)
        xsel = x[cols][:, fs]
        xp16 = np.ascontiguousarray(
            xsel.transpose(1, 3, 2, 0)).reshape(16, T, 512)
        xpk = np.empty((XROWS, T, 512), np.float32)
        xpk[0:16] = xp16
        xpk[16] = (np.arange(T)[:, None] >= lens[cols][None, :]) * MASK_VAL
        xpk[17] = 1.0
        m = dict(tabs[fgrp])
        m["xp"] = xpk.astype(BF16NP)
        in_maps.append(m)
    return in_maps, dict(halves=halves, widths=widths)


def _postprocess(results, meta, inputs):
    W_sq = np.asarray(inputs["W_sq"], np.float32)
    b_sq = np.asarray(inputs["b_sq"], np.float32)
    W_out = np.asarray(inputs["W_out"], np.float32)
    b_out = np.asarray(inputs["b_out"], np.float32)
    halves = meta["halves"]
    h_cat = np.zeros((B, F * HD), np.float32)
    for c in range(N_CORES):
        fgrp, bhalf = c % 4, c // 4
        cols = halves[bhalf]
        h = np.asarray(results[c]["hout"], np.float32) - 1.0
        for fl in range(4):
            f = 4 * fgrp + fl
            h_cat[cols, f * HD:(f + 1) * HD] = h[32 * fl:32 * fl + 32, :].T
    z1 = np.maximum(h_cat @ W_sq.T + b_sq, 0.0)
    out = 1.0 / (1.0 + np.exp(-(z1 @ W_out.T + b_out)))
    return out[:, 0].astype(np.float32)


_CACHE = {}


def _get_compiled(lens):
    key = tuple(np.asarray(lens).tolist())
    hit = _CACHE.get(key)
    if hit is None:
        lens64 = np.asarray(lens).astype(np.int64)
        order = np.argsort(-lens64, kind="stable")
        halves = [order[0::2], order[1::2]]
        halves = [np.concatenate([h[m::MU] for m in range(MU)])
                  for h in halves]
        b_mu = 512 // MU
        widths = []
        for t in range(T):
            k = max(int((lens64[h[0:b_mu]] > t).sum()) for h in halves)
            widths.append(min(b_mu, ((k + 7) // 8) * 8))
        nc = _build_nc(widths)
        _CACHE.clear()          # one compiled program at a time
        _CACHE[key] = nc
        hit = nc
    return hit


def kernel(input, lens, W_ih, W_hh, b_ih, b_hh, W_sq, b_sq, W_out, b_out):
    inputs = dict(input=input, lens=lens, W_ih=W_ih, W_hh=W_hh, b_ih=b_ih,
                  b_hh=b_hh, W_sq=W_sq, b_sq=b_sq, W_out=W_out, b_out=b_out)
    nc = _get_compiled(lens)
    in_maps, meta = _prep(inputs)
    res = bass_utils.run_bass_kernel_spmd(nc, in_maps,
                                          core_ids=list(range(N_CORES)))
    return _postprocess(res.results, meta, inputs)


# revision 3
# speedup vs baseline: 1.7211x; 1.7211x over previous
"""nn_MC_GRU Trainium2 kernel: 8-core SPMD Bass/Tile implementation.

Sharding: 4 feature-groups x 2 batch-halves (one 4-feature group of
independent GRUs per core, 512 batch rows per core). Layout on device is
[128 partitions = 4 features x 32 hidden, B free]. Per timestep, three
matmul-pairs (bf16 inputs, fp32 psum) produce the gate preactivations
(biases folded in via an ones-row in the packed input; the tanh gate is
computed as 2*sigmoid(2x)-1 with the 2x folded into the weights); both
sigmoids run in a single ACT op; the cell update runs on DVE/GPSIMD.

Ragged sequences: the batch is sorted by length (descending) and dealt
across cores/chunks, and the compiled schedule processes only the active
prefix at each step (widths derived from the actual lens at first call —
the program is rebuilt if lens change). Rows past their length are
additionally frozen exactly by a mask row that drives the update-gate
sigmoid to 0. State H = h+1 in fp32 with a bf16 shadow feeding matmuls.

The tiny MLP head (0.5% of FLOPs) runs on host. Output matches the
reference within ~5e-3 relative error (bf16 matmul rounding).
"""

import numpy as np
import ml_dtypes

from concourse import bacc, tile, mybir, bass_utils

F, D, HD, B, T = 16, 4, 32, 1024, 512
N_CORES = 8
MASK_VAL = 88.0
XROWS = 18
MU = 2
CH = 32

f32 = mybir.dt.float32
bf16 = mybir.dt.bfloat16
AF = mybir.ActivationFunctionType
OP = mybir.AluOpType
BF16NP = ml_dtypes.bfloat16


def _build_nc(widths, mu=MU, ch=CH):
    b_mu = 512 // mu
    T_steps = len(widths)
    mmdt = bf16
    nc = bacc.Bacc("TRN2", target_bir_lowering=False, debug=False,
                   num_devices=N_CORES)
    xp = nc.dram_tensor("xp", [XROWS, T, 512], mmdt, kind="ExternalInput").ap()
    drs = {}
    for nm in ("lr", "lzb", "ln2"):
        drs[nm] = nc.dram_tensor(nm, [128, 128], mmdt, kind="ExternalInput").ap()
    for nm in ("gr", "gzb", "gn2"):
        drs[nm] = nc.dram_tensor(nm, [XROWS, 128], mmdt, kind="ExternalInput").ap()
    bhn2 = nc.dram_tensor("bhn2", [128, 1], f32, kind="ExternalInput").ap()
    hout = nc.dram_tensor("hout", [128, 512], f32, kind="ExternalOutput").ap()

    n_chunks = (T_steps + ch - 1) // ch
    with tile.TileContext(nc) as tc:
        with (
            tc.tile_pool(name="wpool", bufs=1) as wpool,
            tc.tile_pool(name="hpool", bufs=1) as hpool,
            tc.tile_pool(name="xpool", bufs=2) as xpool,
            tc.tile_pool(name="work", bufs=2) as work,
            tc.tile_pool(name="psum", bufs=2, space="PSUM") as pp,
        ):
            wL = {}
            for nm, shp in (("lr", [128, 128]), ("lzb", [128, 128]),
                            ("ln2", [128, 128]), ("gr", [XROWS, 128]),
                            ("gzb", [XROWS, 128]), ("gn2", [XROWS, 128])):
                t_ = wpool.tile(shp, mmdt, tag=nm, name=nm)
                nc.sync.dma_start(t_[:], drs[nm][:])
                wL[nm] = t_
            t_ = wpool.tile([128, 1], f32, tag="bhn2", name="bhn2")
            nc.sync.dma_start(t_[:], bhn2[:])
            wL["bhn2"] = t_

            Hm32 = [hpool.tile([128, b_mu], f32, tag=f"Hm{m}", name=f"Hm{m}")
                    for m in range(mu)]
            Hsh = [hpool.tile([128, b_mu], mmdt, tag=f"Hb{m}", name=f"Hb{m}")
                   for m in range(mu)]
            for m in range(mu):
                nc.vector.memset(Hm32[m][:], 1.0)
                nc.vector.memset(Hsh[m][:], 1.0)

            for c in range(n_chunks):
                t0 = c * ch
                t1 = min(t0 + ch, T_steps)
                xt_tile = xpool.tile([XROWS, t1 - t0, 512], mmdt, tag="x",
                                     name="xt_tile")
                nc.sync.dma_start(xt_tile[:], xp[:, t0:t1, :])
                for t in range(t0, t1):
                    w = widths[t]
                    if w <= 0:
                        continue
                    P, WK = [], []
                    for m in range(mu):
                        p_all = pp.tile([128, 4 * b_mu], f32, tag=f"pa{m}",
                                        name=f"pa{m}")
                        P.append(dict(
                            r=p_all[:, 0:w],
                            zb=p_all[:, w:2 * w],
                            rzb=p_all[:, 0:2 * w],
                            hn=p_all[:, 2 * b_mu:2 * b_mu + w],
                            inn=p_all[:, 3 * b_mu:3 * b_mu + w]))
                        WK.append({nm: work.tile(
                            [128, 2 * b_mu] if nm == "srzb" else [128, b_mu],
                            f32, tag=f"{nm}{m}", name=f"{nm}{m}")
                            for nm in ("srzb", "u", "sn", "p1", "g", "v")})
                    for m in range(mu):
                        xsl = xt_tile[:, t - t0, m * b_mu:m * b_mu + w]
                        nc.tensor.matmul(P[m]["inn"], wL["gn2"][:], xsl,
                                         start=True, stop=True)
                        nc.tensor.matmul(P[m]["r"], wL["gr"][:], xsl,
                                         start=True, stop=False)
                        nc.tensor.matmul(P[m]["zb"], wL["gzb"][:], xsl,
                                         start=True, stop=False)
                    for m in range(mu):
                        Hsl = Hsh[m][:, 0:w]
                        nc.tensor.matmul(P[m]["r"], wL["lr"][:], Hsl,
                                         start=False, stop=True)
                        nc.tensor.matmul(P[m]["zb"], wL["lzb"][:], Hsl,
                                         start=False, stop=True)
                        nc.tensor.matmul(P[m]["hn"], wL["ln2"][:], Hsl,
                                         start=True, stop=True)
                    for m in range(mu):
                        nc.scalar.activation(WK[m]["srzb"][:, 0:2 * w],
                                             P[m]["rzb"], AF.Sigmoid)
                    for m in range(mu):
                        nc.vector.scalar_tensor_tensor(
                            WK[m]["u"][:, 0:w], P[m]["hn"], wL["bhn2"][:],
                            WK[m]["srzb"][:, 0:w], op0=OP.add, op1=OP.mult)
                    for m in range(mu):
                        # on DVE: keeps the g+v -> Hsh chain tail same-engine
                        nc.vector.tensor_tensor(
                            WK[m]["p1"][:, 0:w], WK[m]["srzb"][:, w:2 * w],
                            Hm32[m][:, 0:w], op=OP.mult)
                    for m in range(mu):
                        nc.vector.tensor_tensor(
                            P[m]["inn"], WK[m]["u"][:, 0:w],
                            P[m]["inn"], op=OP.add)
                    for m in range(mu):
                        nc.vector.tensor_tensor(
                            WK[m]["g"][:, 0:w], Hm32[m][:, 0:w],
                            WK[m]["p1"][:, 0:w], op=OP.subtract)
                    for m in range(mu):
                        nc.scalar.activation(WK[m]["sn"][:, 0:w],
                                             P[m]["inn"], AF.Sigmoid)
                    for m in range(mu):
                        nc.vector.scalar_tensor_tensor(
                            WK[m]["v"][:, 0:w], WK[m]["sn"][:, 0:w], 2.0,
                            WK[m]["srzb"][:, w:2 * w], op0=OP.mult,
                            op1=OP.mult)
                    for m in range(mu):
                        nc.vector.tensor_tensor(
                            Hsh[m][:, 0:w], WK[m]["g"][:, 0:w],
                            WK[m]["v"][:, 0:w], op=OP.add)
                    for m in range(mu):
                        nc.gpsimd.tensor_tensor(
                            Hm32[m][:, 0:w], WK[m]["g"][:, 0:w],
                            WK[m]["v"][:, 0:w], op=OP.add)

            for m in range(mu):
                nc.sync.dma_start(hout[:, m * b_mu:(m + 1) * b_mu],
                                  Hm32[m][:])
    nc.compile()
    return nc


def _prep(inputs, mu=MU):
    x = np.asarray(inputs["input"], np.float32)
    lens = np.asarray(inputs["lens"]).astype(np.int64)
    W_ih = np.asarray(inputs["W_ih"], np.float32)
    W_hh = np.asarray(inputs["W_hh"], np.float32)
    b_ih = np.asarray(inputs["b_ih"], np.float32)
    b_hh = np.asarray(inputs["b_hh"], np.float32)

    order = np.argsort(-lens, kind="stable")
    halves = [order[0::2], order[1::2]]
    halves = [np.concatenate([h[m::mu] for m in range(mu)]) for h in halves]

    b_mu = 512 // mu
    widths = []
    for t in range(T):
        k = max(int((lens[h[0:b_mu]] > t).sum()) for h in halves)
        widths.append(min(b_mu, ((k + 7) // 8) * 8))

    tabs = {}
    for fgrp in range(4):
        fs = np.arange(4 * fgrp, 4 * fgrp + 4)
        L_r = np.zeros((128, 128), np.float32)
        L_zb = np.zeros((128, 128), np.float32)
        L_n2 = np.zeros((128, 128), np.float32)
        G_r = np.zeros((XROWS, 128), np.float32)
        G_zb = np.zeros((XROWS, 128), np.float32)
        G_n2 = np.zeros((XROWS, 128), np.float32)
        b_hn2 = np.zeros((128, 1), np.float32)
        for fl, f in enumerate(fs):
            sl = slice(32 * fl, 32 * fl + 32)
            rsl = slice(4 * fl, 4 * fl + 4)
            Wr, Wz, Wn = (W_hh[f, 0:32], W_hh[f, 32:64], W_hh[f, 64:96])
            Ur, Uz, Un = (W_ih[f, 0:32], W_ih[f, 32:64], W_ih[f, 64:96])
            L_r[sl, sl] = Wr.T
            L_zb[sl, sl] = -Wz.T
            L_n2[sl, sl] = 2.0 * Wn.T
            G_r[rsl, sl] = Ur.T
            G_zb[rsl, sl] = -Uz.T
            G_n2[rsl, sl] = 2.0 * Un.T
            G_zb[16, sl] = -1.0
            G_r[17, sl] = b_ih[f, 0:32] + b_hh[f, 0:32] - Wr.sum(1)
            G_zb[17, sl] = -(b_ih[f, 32:64] + b_hh[f, 32:64]) + Wz.sum(1)
            G_n2[17, sl] = 2.0 * b_ih[f, 64:96]
            b_hn2[sl, 0] = 2.0 * (b_hh[f, 64:96] - Wn.sum(1))
        tabs[fgrp] = dict(lr=L_r.astype(BF16NP), lzb=L_zb.astype(BF16NP),
                          ln2=L_n2.astype(BF16NP), gr=G_r.astype(BF16NP),
                          gzb=G_zb.astype(BF16NP), gn2=G_n2.astype(BF16NP),
                          bhn2=b_hn2)

    in_maps = []
    for c in range(N_CORES):
        fgrp, bhalf = c % 4, c // 4
        cols = halves[bhalf]
        fs = np.arange(4 * fgrp, 4 * fgrp + 4)
        xsel = x[cols][:, fs]
        xp16 = np.ascontiguousarray(
            xsel.transpose(1, 3, 2, 0)).reshape(16, T, 512)
        xpk = np.empty((XROWS, T, 512), np.float32)
        xpk[0:16] = xp16
        xpk[16] = (np.arange(T)[:, None] >= lens[cols][None, :]) * MASK_VAL
        xpk[17] = 1.0
        m = dict(tabs[fgrp])
        m["xp"] = xpk.astype(BF16NP)
        in_maps.append(m)
    return in_maps, dict(halves=halves, widths=widths)


def _postprocess(results, meta, inputs):
    W_sq = np.asarray(inputs["W_sq"], np.float32)
    b_sq = np.asarray(inputs["b_sq"], np.float32)
    W_out = np.asarray(inputs["W_out"], np.float32)
    b_out = np.asarray(inputs["b_out"], np.float32)
    halves = meta["halves"]
    h_cat = np.zeros((B, F * HD), np.float32)
    for c in range(N_CORES):
        fgrp, bhalf = c % 4, c // 4
        cols = halves[bhalf]
        h = np.asarray(results[c]["hout"], np.float32) - 1.0
        for fl in range(4):
            f = 4 * fgrp + fl
            h_cat[cols, f * HD:(f + 1) * HD] = h[32 * fl:32 * fl + 32, :].T
    z1 = np.maximum(h_cat @ W_sq.T + b_sq, 0.0)
    out = 1.0 / (1.0 + np.exp(-(z1 @ W_out.T + b_out)))
    return out[:, 0].astype(np.float32)


_CACHE = {}


def _get_compiled(lens):
    key = tuple(np.asarray(lens).tolist())
    hit = _CACHE.get(key)
    if hit is None:
        lens64 = np.asarray(lens).astype(np.int64)
        order = np.argsort(-lens64, kind="stable")
        halves = [order[0::2], order[1::2]]
        halves = [np.concatenate([h[m::MU] for m in range(MU)])
                  for h in halves]
        b_mu = 512 // MU
        widths = []
        for t in range(T):
            k = max(int((lens64[h[0:b_mu]] > t).sum()) for h in halves)
            widths.append(min(b_mu, ((k + 7) // 8) * 8))
        nc = _build_nc(widths)
        _CACHE.clear()          # one compiled program at a time
        _CACHE[key] = nc
        hit = nc
    return hit


def kernel(input, lens, W_ih, W_hh, b_ih, b_hh, W_sq, b_sq, W_out, b_out):
    inputs = dict(input=input, lens=lens, W_ih=W_ih, W_hh=W_hh, b_ih=b_ih,
                  b_hh=b_hh, W_sq=W_sq, b_sq=b_sq, W_out=W_out, b_out=b_out)
    nc = _get_compiled(lens)
    in_maps, meta = _prep(inputs)
    res = bass_utils.run_bass_kernel_spmd(nc, in_maps,
                                          core_ids=list(range(N_CORES)))
    return _postprocess(res.results, meta, inputs)
